# revision 10
# baseline (speedup 1.0000x reference)
"""Multi-head latent attention kernel for Trainium2, 8 NeuronCores.

Problem (hardcoded shapes):
  hidden_states [2, 2048, 4096] f32, attention_mask [1,1,2048,2048] f32,
  Wq [4096,4096], Wk/Wv [4096,1024], Wo [4096,4096].
  4 query heads x 1024 head_dim, 1 kv head, interleaved RoPE, softmax, o-proj.

Sharding: core c = (batch b=c//4, r=c%4), all within-batch groups of 4.
  - k^T / v computed from the core's sequence quarter and AllGathered in two
    separate collectives (k right after the K pass, v after the V pass) so
    both overlap the Q projection.
  - Attention is sharded over QUERY positions at 128-row granularity: core r
    handles global 128-row i-blocks {4k + r} for slot k in 0..3 -- slot k's
    union key range is jc < 4k+4, identical for every core, so the SPMD
    program wastes only the 128-col mask wedge instead of whole 256-row
    blocks. Scores/exp/PV use ragged suffix tiles: for key chunk jc the
    moving operand is qT[:, 128*s(jc):512] (s = first live slot), and the
    denominator / PV accumulations walk jc ascending so PSUM regions shrink
    inside one accumulation group (first matmul zero-initializes the full
    bank). The o-projection stays fully local (no second collective).
  - DMA queue discipline: weights (wk/wv/wq/wo) + cos/sin + initial hsq
    chunks ride the Sync ring; hqc2 (WAR-gated reloads) and output stores
    ride the Vector ring; k/v bounce stores, both AllGathers, kT/vT/mask
    loads ride the GpSimd ring. No consumer ever queues behind a slow
    producer on the same FIFO ring.
All matmul operands bf16, f32 PSUM accumulation. RoPE is a host-side
deinterleave permutation of Wq/Wk columns plus 6 elementwise ops per
(even,odd) chunk pair against cos/sin tables. The mask is handled
generically per (jc, slot): skip / clean / mixed with per-core
(1/SCALE)-prescaled 128x128 mask tiles, so causal, zero, and arbitrary
additive masks are all supported.
"""

import numpy as np
import ml_dtypes

from concourse import bass, mybir, tile, bacc
from concourse import bass_utils

BF16 = mybir.dt.bfloat16
F32 = mybir.dt.float32

B, S, H = 2, 2048, 4096
NH, D = 4, 1024  # query heads, head dim
PD = D // 2  # rope pair count (512)
SCALE = D ** -0.5
NCORES = 8
GROUPS = [[0, 1, 2, 3], [4, 5, 6, 7]]

KC = H // 128  # 32 contraction chunks over hidden
DC = D // 128  # 8 d-chunks of head dim
JC = S // 128  # 16 j-chunks (key) of 128
Q = S // 4  # 512, per-core kv sequence quarter

# results of the traced+profiled run (filled by kernel() when trace=True)
LAST_RESULTS = None

PAIRS = [(0, 4), (1, 5), (2, 6), (3, 7)]


def _build(slot_start, cats, n_mixed):
    """Build the SPMD bass program.

    slot_start[jc]: first live slot s for key chunk jc, or -1 to skip jc.
    cats[(jc, k)] for k in s(jc)..3: "clean" | int (packed mask tile index).
    """
    nc = bacc.Bacc("TRN2", target_bir_lowering=False, debug=False,
                   num_devices=NCORES)

    hsq_d = nc.dram_tensor("hsq", [H, Q], BF16, kind="ExternalInput")
    hsq2_d = nc.dram_tensor("hsq2", [H, 512], BF16, kind="ExternalInput")
    wq_d = nc.dram_tensor("wq", [H, H], BF16, kind="ExternalInput")
    wk_d = nc.dram_tensor("wk", [H, D], BF16, kind="ExternalInput")
    wv_d = nc.dram_tensor("wv", [H, D], BF16, kind="ExternalInput")
    wo_d = nc.dram_tensor("wo", [H, H], BF16, kind="ExternalInput")
    cosq_d = nc.dram_tensor("cosq", [PD, Q], BF16, kind="ExternalInput")
    sinq_d = nc.dram_tensor("sinq", [PD, Q], BF16, kind="ExternalInput")
    cosq2_d = nc.dram_tensor("cosq2", [PD, 512], BF16, kind="ExternalInput")
    sinq2_d = nc.dram_tensor("sinq2", [PD, 512], BF16, kind="ExternalInput")
    nmask = max(n_mixed, 1)
    maskp_d = nc.dram_tensor("maskp", [nmask * 128, 128], F32,
                             kind="ExternalInput")
    out_d = nc.dram_tensor("out", [512, H], F32, kind="ExternalOutput")

    # collective bounce buffers: k^T slice [1024 d, 512 s], v slice flattened
    # row-major to [1024, 512] (dram row 2p+dvb = s-row p, d half dvb)
    k_in = nc.dram_tensor("k_in", [1024, 512], BF16, kind="Internal")
    k_out = nc.dram_tensor("k_out", [4096, 512], BF16, kind="Internal")
    v_in = nc.dram_tensor("v_in", [1024, 512], BF16, kind="Internal")
    v_out = nc.dram_tensor("v_out", [4096, 512], BF16, kind="Internal")

    live_jc = [jc for jc in range(JC) if slot_start[jc] >= 0]
    smin = min(slot_start[jc] for jc in live_jc) if live_jc else 0

    with tile.TileContext(nc) as tc:
        with tc.tile_pool(name="pers", bufs=1) as pers:
            ones_col = pers.tile([128, 1], BF16, name="ones_col",
                                 tag="ones_col")
            nc.vector.memset(ones_col[:], 1.0)
            ones_row = pers.tile([1, 128], F32, name="ones_row",
                                 tag="ones_row")
            nc.vector.memset(ones_row[:], 1.0)
            # q^T for all 4 heads, local i columns: 32 chunks [128 d, 512 i]
            qT = [pers.tile([128, 512], BF16, name=f"qt{i}", tag=f"qt{i}")
                  for i in range(4 * DC)]
            # k^T full sequence: 8 chunks [128 d, 2048 j]
            kT = [pers.tile([128, S], BF16, name=f"kt{i}", tag=f"kt{i}")
                  for i in range(DC)]
            # v rows per 128-key chunk: 16 tiles [128 j, 1024 d]
            vT = [pers.tile([128, D], BF16, name=f"vt{i}", tag=f"vt{i}")
                  for i in range(JC)]
            maskt = [pers.tile([128, 128], F32, name=f"mk{i}", tag=f"mk{i}")
                     for i in range(n_mixed)]

            # ============ phase A: projections + k/v AllGathers ============
            with (
                tc.tile_pool(name="pa", bufs=3) as pa,
                tc.tile_pool(name="paps", bufs=8, space="PSUM") as paps,
            ):
                # interleave first wk tiles with fine-grained hqc chunk loads
                # so the K pass starts after ~0.4MB instead of 4MB
                wkts = []
                for kc in range(2):
                    wkt = pa.tile([128, D], BF16, name="wk", tag="wk", bufs=4)
                    nc.sync.dma_start(wkt[:], wk_d[128 * kc:128 * (kc + 1), :])
                    wkts.append(wkt)
                hqc = [pa.tile([128, 8, Q], BF16, name=f"hqc{i}",
                               tag=f"hqc{i}", bufs=1) for i in range(4)]
                for i in range(4):
                    for c in range(8):
                        nc.sync.dma_start(
                            hqc[i][:, c, :],
                            hsq_d[1024 * i + 128 * c:
                                  1024 * i + 128 * (c + 1), :])

                def rope_pair(ps_e, ps_o, c_t, s_t, out_e, out_o, n):
                    """Stage psum pair to bf16, apply rope, write outputs."""
                    st_e = pa.tile([128, n], BF16, name="stg", tag="stg",
                                   bufs=6)
                    st_o = pa.tile([128, n], BF16, name="stg", tag="stg",
                                   bufs=6)
                    nc.scalar.activation(st_e[:], ps_e[:],
                                         mybir.ActivationFunctionType.Copy)
                    nc.scalar.activation(st_o[:], ps_o[:],
                                         mybir.ActivationFunctionType.Copy)
                    t1 = pa.tile([128, n], BF16, name="rtmp", tag="rtmp",
                                 bufs=4)
                    t2 = pa.tile([128, n], BF16, name="rtmp", tag="rtmp",
                                 bufs=4)
                    nc.vector.tensor_mul(t1[:], st_e[:], c_t)
                    nc.vector.tensor_mul(t2[:], st_o[:], s_t)
                    nc.vector.tensor_sub(out_e, t1[:], t2[:])
                    t3 = pa.tile([128, n], BF16, name="rtmp", tag="rtmp",
                                 bufs=4)
                    t4 = pa.tile([128, n], BF16, name="rtmp", tag="rtmp",
                                 bufs=4)
                    nc.vector.tensor_mul(t3[:], st_o[:], c_t)
                    nc.vector.tensor_mul(t4[:], st_e[:], s_t)
                    nc.vector.tensor_add(out_o, t3[:], t4[:])

                # --- K pass: kc-outer over 8 psum banks, then rope pairs ---
                kps = [paps.tile([128, Q], F32, name="mmps", tag="mmps")
                       for _ in range(DC)]
                for kc in range(KC):
                    if kc < 2:
                        wkt = wkts[kc]
                    else:
                        wkt = pa.tile([128, D], BF16, name="wk", tag="wk",
                                      bufs=4)
                        nc.sync.dma_start(wkt[:],
                                          wk_d[128 * kc:128 * (kc + 1), :])
                    for dc in range(DC):
                        nc.tensor.matmul(
                            kps[dc][:], wkt[:, 128 * dc:128 * (dc + 1)],
                            hqc[kc // 8][:, kc % 8, :],
                            start=(kc == 0), stop=(kc == KC - 1))
                for pi, (de, do) in enumerate(PAIRS):
                    c_t = pa.tile([128, Q], BF16, name="ckt", tag="ckt",
                                  bufs=2)
                    s_t = pa.tile([128, Q], BF16, name="skt", tag="skt",
                                  bufs=2)
                    nc.sync.dma_start(c_t[:],
                                      cosq_d[128 * pi:128 * (pi + 1), :])
                    nc.sync.dma_start(s_t[:],
                                      sinq_d[128 * pi:128 * (pi + 1), :])
                    ke = pa.tile([128, Q], BF16, name="kout", tag="kout",
                                 bufs=4)
                    ko = pa.tile([128, Q], BF16, name="kout", tag="kout",
                                 bufs=4)
                    rope_pair(kps[de], kps[do], c_t[:], s_t[:], ke[:], ko[:],
                              Q)
                    nc.gpsimd.dma_start(k_in[128 * de:128 * (de + 1), :],
                                        ke[:])
                    nc.gpsimd.dma_start(k_in[128 * do:128 * (do + 1), :],
                                        ko[:])

                # --- k AllGather (overlaps V + Q passes) ---
                nc.gpsimd.collective_compute(
                    "AllGather", mybir.AluOpType.bypass, replica_groups=GROUPS,
                    ins=[k_in.ap().opt()], outs=[k_out.ap().opt()])

                # --- V pass: kc-outer over 8 psum banks ---
                vps = [paps.tile([128, 512], F32, name="mmps", tag="mmps")
                       for _ in range(8)]
                for kc in range(KC):
                    wvt = pa.tile([128, D], BF16, name="wv", tag="wv", bufs=4)
                    nc.sync.dma_start(wvt[:], wv_d[128 * kc:128 * (kc + 1), :])
                    for sc in range(4):
                        for dvb in range(2):
                            nc.tensor.matmul(
                                vps[sc * 2 + dvb][:],
                                hqc[kc // 8][:, kc % 8, 128 * sc:128 * (sc + 1)],
                                wvt[:, 512 * dvb:512 * (dvb + 1)],
                                start=(kc == 0), stop=(kc == KC - 1))
                # hqc2 reuses the hqc ring slots (WAR releases as the V pass
                # drains each hqc tile); the scalar ring is idle here, so
                # these transfers never block the sync (weight) ring
                hqc2 = [pa.tile([128, 8, 512], BF16, name=f"hqc{i}",
                                tag=f"hqc{i}", bufs=1) for i in range(4)]
                for i in range(4):
                    for c in range(8):
                        nc.scalar.dma_start(
                            hqc2[i][:, c, :],
                            hsq2_d[1024 * i + 128 * c:
                                   1024 * i + 128 * (c + 1), :])
                for sc in range(4):
                    for dvb in range(2):
                        vt = pa.tile([128, 512], BF16, name="vout", tag="vout",
                                     bufs=4)
                        nc.scalar.activation(vt[:], vps[sc * 2 + dvb][:],
                                             mybir.ActivationFunctionType.Copy)
                        dst = v_in[256 * sc:256 * (sc + 1), :]
                        dst = dst.rearrange("(p c) f -> p c f", c=2)[:, dvb, :]
                        nc.gpsimd.dma_start(dst, vt[:])

                # --- v AllGather (overlaps Q pass) ---
                nc.gpsimd.collective_compute(
                    "AllGather", mybir.AluOpType.bypass, replica_groups=GROUPS,
                    ins=[v_in.ap().opt()], outs=[v_out.ap().opt()])

                # phase-B inputs on the gpsimd ring: wait on the AGs without
                # ever blocking the sync (weight) ring
                for dc in range(DC):
                    for r2 in range(4):
                        nc.gpsimd.dma_start(
                            kT[dc][:, Q * r2:Q * (r2 + 1)],
                            k_out[1024 * r2 + 128 * dc:
                                  1024 * r2 + 128 * (dc + 1), :])
                for jc in range(JC):
                    base = 1024 * (jc // 4) + 256 * (jc % 4)
                    nc.gpsimd.dma_start(
                        vT[jc][:],
                        v_out[base:base + 256, :].rearrange(
                            "(p c) f -> p (c f)", c=2))
                for mi in range(n_mixed):
                    nc.gpsimd.dma_start(
                        maskt[mi][:], maskp_d[128 * mi:128 * (mi + 1), :])

                # --- Q pass: all 4 heads x 8 d-chunks over local i cols ---
                for hp in range(4):
                    qps = [paps.tile([128, 512], F32, name="mmps", tag="mmps")
                           for _ in range(DC)]
                    for kc in range(KC):
                        wqt = pa.tile([128, D], BF16, name="wqs", tag="wqs",
                                      bufs=12)
                        nc.sync.dma_start(
                            wqt[:], wq_d[128 * kc:128 * (kc + 1),
                                         D * hp:D * (hp + 1)])
                        for dc in range(DC):
                            nc.tensor.matmul(
                                qps[dc][:], wqt[:, 128 * dc:128 * (dc + 1)],
                                hqc2[kc // 8][:, kc % 8, :],
                                start=(kc == 0), stop=(kc == KC - 1))
                    for pi, (de, do) in enumerate(PAIRS):
                        c_t = pa.tile([128, 512], BF16, name="cqt", tag="cqt",
                                      bufs=3)
                        s_t = pa.tile([128, 512], BF16, name="sqt", tag="sqt",
                                      bufs=3)
                        nc.sync.dma_start(c_t[:],
                                          cosq2_d[128 * pi:128 * (pi + 1), :])
                        nc.sync.dma_start(s_t[:],
                                          sinq2_d[128 * pi:128 * (pi + 1), :])
                        rope_pair(qps[de], qps[do], c_t[:], s_t[:],
                                  qT[DC * hp + de][:], qT[DC * hp + do][:],
                                  512)

            # ====== phase B: attention, ragged suffix tiles per jc ======
            # accumulation chains must start with the widest extent so the
            # start matmul zero-initializes every region later accumulated
            chain_jc = sorted(live_jc, key=lambda jc: (slot_start[jc], jc))
            ragged = len({slot_start[jc] for jc in live_jc}) > 1
            with tc.tile_pool(name="pb", bufs=2) as pb:
                pbps_cm = tc.tile_pool(name="pbps", bufs=2, space="PSUM")
                pbps = pbps_cm.__enter__()
                # attention output, transposed: 32 chunks [128 hdv, 512 i]
                attnT = [pb.tile([128, 512], BF16, name=f"att{i}",
                                 tag=f"att{i}", bufs=1) for i in range(KC)]
                if smin > 0:
                    for t in attnT:
                        nc.vector.memset(t[:, :128 * smin], 0.0)

                for h in range(NH):
                    pT = {}
                    for jc in live_jc:
                        s = slot_start[jc]
                        w = 512 - 128 * s
                        sps = pbps.tile([128, 512], F32, name="sps",
                                        tag="sps", bufs=3)
                        for dc in range(DC):
                            nc.tensor.matmul(
                                sps[:, :w],
                                kT[dc][:, 128 * jc:128 * (jc + 1)],
                                qT[DC * h + dc][:, 128 * s:512],
                                start=(dc == 0), stop=(dc == DC - 1))
                        for k in range(s, 4):
                            cat = cats[(jc, k)]
                            if isinstance(cat, int):
                                off = 128 * (k - s)
                                nc.vector.tensor_add(
                                    sps[:, off:off + 128],
                                    sps[:, off:off + 128], maskt[cat][:])
                        pt = pb.tile([128, w], BF16, name=f"pt{jc}",
                                     tag=f"pt{jc}", bufs=2)
                        nc.scalar.activation(
                            pt[:], sps[:, :w],
                            mybir.ActivationFunctionType.Exp, scale=SCALE)
                        pT[jc] = pt
                    # denominator: widest-first chain => shrinking psum extents
                    l_ps = pbps.tile([1, 512], F32, name="lps", tag="lps",
                                     bufs=1)
                    for n, jc in enumerate(chain_jc):
                        s = slot_start[jc]
                        nc.tensor.matmul(l_ps[:, 128 * s:512], ones_col[:],
                                         pT[jc][:], start=(n == 0),
                                         stop=(n == len(chain_jc) - 1),
                                         skip_group_check=ragged)
                    r_sb = pb.tile([1, 512], F32, name="rsb", tag="rsb",
                                   bufs=2)
                    nc.vector.reciprocal(r_sb[:, 128 * smin:512],
                                         l_ps[:, 128 * smin:512])
                    r_ps = pbps.tile([128, 512], F32, name="rps",
                                     tag="rps", bufs=1)
                    nc.tensor.matmul(r_ps[:, 128 * smin:512], ones_row[:],
                                     r_sb[:, 128 * smin:512],
                                     start=True, stop=True)
                    rbc = pb.tile([128, 512], F32, name="rbc", tag="rbc",
                                  bufs=2)
                    nc.scalar.activation(rbc[:, 128 * smin:512],
                                         r_ps[:, 128 * smin:512],
                                         mybir.ActivationFunctionType.Copy)
                    for dc2 in range(DC):
                        pvps = pbps.tile([128, 512], F32, name="pvps",
                                         tag="pvps", bufs=2)
                        for n, jc in enumerate(chain_jc):
                            s = slot_start[jc]
                            nc.tensor.matmul(
                                pvps[:, 128 * s:512],
                                vT[jc][:, 128 * dc2:128 * (dc2 + 1)],
                                pT[jc][:], start=(n == 0),
                                stop=(n == len(chain_jc) - 1),
                                skip_group_check=ragged)
                        nc.vector.tensor_mul(
                            attnT[DC * h + dc2][:, 128 * smin:512],
                            pvps[:, 128 * smin:512],
                            rbc[:, 128 * smin:512])

                pbps_cm.__exit__(None, None, None)

                # ============ phase C: local output projection ============
                with (
                    tc.tile_pool(name="pc", bufs=2) as pc,
                    tc.tile_pool(name="pcps", bufs=4, space="PSUM") as pcps,
                ):
                    for eb in range(8):
                        ops = [pcps.tile([128, 512], F32, name="ops",
                                         tag="ops") for _ in range(4)]
                        for kc in range(KC):
                            wot = pc.tile([128, 512], BF16, name="wot",
                                          tag="wot", bufs=8)
                            nc.sync.dma_start(
                                wot[:], wo_d[128 * kc:128 * (kc + 1),
                                             512 * eb:512 * (eb + 1)])
                            for ic in range(4):
                                nc.tensor.matmul(
                                    ops[ic][:],
                                    attnT[kc][:, 128 * ic:128 * (ic + 1)],
                                    wot[:], start=(kc == 0),
                                    stop=(kc == KC - 1))
                        for ic in range(4):
                            ot = pc.tile([128, 512], F32, name="otile",
                                         tag="otile", bufs=4)
                            nc.vector.tensor_copy(ot[:], ops[ic][:])
                            nc.scalar.dma_start(
                                out_d[128 * ic:128 * (ic + 1),
                                      512 * eb:512 * (eb + 1)], ot[:])

    nc.compile()
    return nc


_BUILD_CACHE = {}


def _classify_mask(mask):
    """Slot-classify each (jc, slot k) over the 4 quarter cores.

    Core r owns global 128-row i-blocks {4k + r}, k in 0..3. Slot k of key
    chunk jc is dead if every core's block is fully masked, clean if every
    core's block is all-zero, else mixed (packed per-core mask tile).
    Returns (slot_start, cats, per-core packed [n*128,128] arrays, n_mixed).
    """
    m = np.asarray(mask).reshape(S, S)  # [i, j]
    slot_start = {}
    cats = {}
    tiles = [[] for _ in range(4)]
    n = 0
    for jc in range(JC):
        regs = {}
        for k in range(4):
            regs[k] = [m[128 * (4 * k + r):128 * (4 * k + r + 1),
                        128 * jc:128 * (jc + 1)] for r in range(4)]
        live_k = [k for k in range(4)
                  if not all(np.all(b <= -1e8) for b in regs[k])]
        if not live_k:
            slot_start[jc] = -1
            continue
        s = min(live_k)
        slot_start[jc] = s
        for k in range(s, 4):
            if not any(b.any() for b in regs[k]):
                cats[(jc, k)] = "clean"
            else:
                cats[(jc, k)] = n
                n += 1
                for r in range(4):
                    # [j, i] orientation, prescaled by 1/SCALE so the ACT's
                    # uniform SCALE reproduces reference's scores*SCALE + mask
                    tiles[r].append(
                        np.ascontiguousarray(regs[k][r].T) * (1.0 / SCALE))
    maskps = [
        np.concatenate(t, axis=0).astype(np.float32) if t
        else np.zeros((128, 128), np.float32) for t in tiles]
    return slot_start, cats, maskps, n


def kernel(hidden_states, attention_mask, Wq, Wk, Wv, Wo, trace=False):
    global LAST_RESULTS
    bf = ml_dtypes.bfloat16

    slot_start, cats, maskps, n_mixed = _classify_mask(attention_mask)
    key = (tuple(sorted(slot_start.items())),
           tuple(sorted((k, v if isinstance(v, str) else "m")
                        for k, v in cats.items())))
    if key not in _BUILD_CACHE:
        _BUILD_CACHE[key] = _build(slot_start, cats, n_mixed)
    nc = _BUILD_CACHE[key]

    # deinterleave rope pairs within each head's 1024 columns
    perm = np.concatenate([np.arange(0, D, 2), np.arange(1, D, 2)])
    cols = np.concatenate([h * D + perm for h in range(NH)])
    wq_p = np.ascontiguousarray(Wq[:, cols]).astype(bf)
    wk_p = np.ascontiguousarray(Wk[:, perm]).astype(bf)
    wv_c = np.ascontiguousarray(Wv).astype(bf)
    wo_c = np.ascontiguousarray(Wo).astype(bf)

    freqs = 1.0 / (10000.0 ** (np.arange(0, D, 2, dtype=np.float64) / D))
    ang = np.outer(np.arange(S, dtype=np.float64), freqs)  # [S, PD]
    cosT = np.ascontiguousarray(np.cos(ang).T).astype(bf)  # [PD, S]
    sinT = np.ascontiguousarray(np.sin(ang).T).astype(bf)

    hsT = [np.ascontiguousarray(hidden_states[b].T).astype(bf)
           for b in range(B)]

    in_maps = []
    for c in range(NCORES):
        b, r = c // 4, c % 4
        icols = np.concatenate(
            [np.arange(128 * (4 * k + r), 128 * (4 * k + r + 1))
             for k in range(4)])
        in_maps.append({
            "hsq": np.ascontiguousarray(hsT[b][:, Q * r:Q * (r + 1)]),
            "hsq2": np.ascontiguousarray(hsT[b][:, icols]),
            "wq": wq_p,
            "wk": wk_p,
            "wv": wv_c,
            "wo": wo_c,
            "cosq": np.ascontiguousarray(cosT[:, Q * r:Q * (r + 1)]),
            "sinq": np.ascontiguousarray(sinT[:, Q * r:Q * (r + 1)]),
            "cosq2": np.ascontiguousarray(cosT[:, icols]),
            "sinq2": np.ascontiguousarray(sinT[:, icols]),
            "maskp": maskps[r],
        })

    res = bass_utils.run_bass_kernel_spmd(
        nc, in_maps, core_ids=list(range(NCORES)), trace=trace)
    LAST_RESULTS = res

    out = np.empty((B, S, H), np.float32)
    for c in range(NCORES):
        b, r = c // 4, c % 4
        o = res.results[c]["out"]
        for k in range(4):
            g = 4 * k + r
            out[b, 128 * g:128 * (g + 1), :] = o[128 * k:128 * (k + 1)]
    return out


# revision 12
# speedup vs baseline: 1.0612x; 1.0612x over previous
"""Multi-head latent attention kernel for Trainium2, 8 NeuronCores.

Problem (hardcoded shapes):
  hidden_states [2, 2048, 4096] f32, attention_mask [1,1,2048,2048] f32,
  Wq [4096,4096], Wk/Wv [4096,1024], Wo [4096,4096].
  4 query heads x 1024 head_dim, 1 kv head, interleaved RoPE, softmax, o-proj.

Sharding: core c = (batch b=c//4, r=c%4), all within-batch groups of 4.
  - k^T / v computed from the core's sequence quarter and AllGathered in two
    separate collectives (k right after the K pass, v after the V pass) so
    both overlap the Q projection.
  - Attention is sharded over QUERY positions at 128-row granularity: core r
    handles global 128-row i-blocks {4k + r} for slot k in 0..3 -- slot k's
    union key range is jc < 4k+4, identical for every core, so the SPMD
    program wastes only the 128-col mask wedge instead of whole 256-row
    blocks. Scores/exp/PV use ragged suffix tiles: for key chunk jc the
    moving operand is qT[:, 128*s(jc):512] (s = first live slot), and the
    denominator / PV accumulations walk jc ascending so PSUM regions shrink
    inside one accumulation group (first matmul zero-initializes the full
    bank). The o-projection stays fully local (no second collective).
  - DMA queue discipline: weights (wk/wv/wq/wo) + cos/sin + initial hsq
    chunks ride the Sync ring; hqc2 (WAR-gated reloads) and output stores
    ride the Vector ring; k/v bounce stores, both AllGathers, kT/vT/mask
    loads ride the GpSimd ring. No consumer ever queues behind a slow
    producer on the same FIFO ring.
All matmul operands bf16, f32 PSUM accumulation. RoPE is a host-side
deinterleave permutation of Wq/Wk columns plus 6 elementwise ops per
(even,odd) chunk pair against cos/sin tables. The mask is handled
generically per (jc, slot): skip / clean / mixed with per-core
(1/SCALE)-prescaled 128x128 mask tiles, so causal, zero, and arbitrary
additive masks are all supported.
"""

import numpy as np
import ml_dtypes

from concourse import bass, mybir, tile, bacc
from concourse import bass_utils

BF16 = mybir.dt.bfloat16
F32 = mybir.dt.float32

B, S, H = 2, 2048, 4096
NH, D = 4, 1024  # query heads, head dim
PD = D // 2  # rope pair count (512)
SCALE = D ** -0.5
NCORES = 8
GROUPS = [[0, 1, 2, 3], [4, 5, 6, 7]]

KC = H // 128  # 32 contraction chunks over hidden
DC = D // 128  # 8 d-chunks of head dim
JC = S // 128  # 16 j-chunks (key) of 128
Q = S // 4  # 512, per-core kv sequence quarter

# results of the traced+profiled run (filled by kernel() when trace=True)
LAST_RESULTS = None

PAIRS = [(0, 4), (1, 5), (2, 6), (3, 7)]


def _build(slot_start, cats, n_mixed):
    """Build the SPMD bass program.

    slot_start[jc]: first live slot s for key chunk jc, or -1 to skip jc.
    cats[(jc, k)] for k in s(jc)..3: "clean" | int (packed mask tile index).
    """
    nc = bacc.Bacc("TRN2", target_bir_lowering=False, debug=False,
                   num_devices=NCORES)

    hsq_d = nc.dram_tensor("hsq", [H, Q], BF16, kind="ExternalInput")
    hsq2_d = nc.dram_tensor("hsq2", [H, 512], BF16, kind="ExternalInput")
    wq_d = nc.dram_tensor("wq", [H, H], BF16, kind="ExternalInput")
    wk_d = nc.dram_tensor("wk", [H, D], BF16, kind="ExternalInput")
    wv_d = nc.dram_tensor("wv", [H, D], BF16, kind="ExternalInput")
    wo_d = nc.dram_tensor("wo", [H, H], BF16, kind="ExternalInput")
    cosq_d = nc.dram_tensor("cosq", [PD, Q], BF16, kind="ExternalInput")
    sinq_d = nc.dram_tensor("sinq", [PD, Q], BF16, kind="ExternalInput")
    cosq2_d = nc.dram_tensor("cosq2", [PD, 512], BF16, kind="ExternalInput")
    sinq2_d = nc.dram_tensor("sinq2", [PD, 512], BF16, kind="ExternalInput")
    nmask = max(n_mixed, 1)
    maskp_d = nc.dram_tensor("maskp", [nmask * 128, 128], F32,
                             kind="ExternalInput")
    out_d = nc.dram_tensor("out", [512, H], F32, kind="ExternalOutput")

    # collective bounce buffers: k^T slice [1024 d, 512 s], v slice flattened
    # row-major to [1024, 512] (dram row 2p+dvb = s-row p, d half dvb)
    k_in = nc.dram_tensor("k_in", [1024, 512], BF16, kind="Internal")
    k_out = nc.dram_tensor("k_out", [4096, 512], BF16, kind="Internal")
    v_in = nc.dram_tensor("v_in", [1024, 512], BF16, kind="Internal")
    v_out = nc.dram_tensor("v_out", [4096, 512], BF16, kind="Internal")

    live_jc = [jc for jc in range(JC) if slot_start[jc] >= 0]
    smin = min(slot_start[jc] for jc in live_jc) if live_jc else 0

    with tile.TileContext(nc) as tc:
        with tc.tile_pool(name="pers", bufs=1) as pers:
            ones_col = pers.tile([128, 1], BF16, name="ones_col",
                                 tag="ones_col")
            nc.vector.memset(ones_col[:], 1.0)
            ones_row = pers.tile([1, 128], F32, name="ones_row",
                                 tag="ones_row")
            nc.vector.memset(ones_row[:], 1.0)
            # q^T for all 4 heads, local i columns: 32 chunks [128 d, 512 i]
            qT = [pers.tile([128, 512], BF16, name=f"qt{i}", tag=f"qt{i}")
                  for i in range(4 * DC)]
            # k^T full sequence: 8 chunks [128 d, 2048 j]
            kT = [pers.tile([128, S], BF16, name=f"kt{i}", tag=f"kt{i}")
                  for i in range(DC)]
            # v rows per 128-key chunk: 16 tiles [128 j, 1024 d]
            vT = [pers.tile([128, D], BF16, name=f"vt{i}", tag=f"vt{i}")
                  for i in range(JC)]
            maskt = [pers.tile([128, 128], F32, name=f"mk{i}", tag=f"mk{i}")
                     for i in range(n_mixed)]

            # ============ phase A: projections + k/v AllGathers ============
            with (
                tc.tile_pool(name="pa", bufs=3) as pa,
                tc.tile_pool(name="paps", bufs=8, space="PSUM") as paps,
            ):
                # interleave first wk tiles with fine-grained hqc chunk loads
                # so the K pass starts after ~0.4MB instead of 4MB
                wkts = []
                for kc in range(2):
                    wkt = pa.tile([128, D], BF16, name="wk", tag="wk", bufs=4)
                    nc.sync.dma_start(wkt[:], wk_d[128 * kc:128 * (kc + 1), :])
                    wkts.append(wkt)
                hqc = [pa.tile([128, 8, Q], BF16, name=f"hqc{i}",
                               tag=f"hqc{i}", bufs=1) for i in range(4)]
                for i in range(4):
                    for c in range(8):
                        nc.sync.dma_start(
                            hqc[i][:, c, :],
                            hsq_d[1024 * i + 128 * c:
                                  1024 * i + 128 * (c + 1), :])

                def rope_pair(ps_e, ps_o, c_t, s_t, out_e, out_o, n):
                    """Stage psum pair to bf16, apply rope, write outputs."""
                    st_e = pa.tile([128, n], BF16, name="stg", tag="stg",
                                   bufs=6)
                    st_o = pa.tile([128, n], BF16, name="stg", tag="stg",
                                   bufs=6)
                    nc.scalar.activation(st_e[:], ps_e[:],
                                         mybir.ActivationFunctionType.Copy)
                    nc.scalar.activation(st_o[:], ps_o[:],
                                         mybir.ActivationFunctionType.Copy)
                    t1 = pa.tile([128, n], BF16, name="rtmp", tag="rtmp",
                                 bufs=4)
                    t2 = pa.tile([128, n], BF16, name="rtmp", tag="rtmp",
                                 bufs=4)
                    nc.vector.tensor_mul(t1[:], st_e[:], c_t)
                    nc.vector.tensor_mul(t2[:], st_o[:], s_t)
                    nc.vector.tensor_sub(out_e, t1[:], t2[:])
                    t3 = pa.tile([128, n], BF16, name="rtmp", tag="rtmp",
                                 bufs=4)
                    t4 = pa.tile([128, n], BF16, name="rtmp", tag="rtmp",
                                 bufs=4)
                    nc.vector.tensor_mul(t3[:], st_o[:], c_t)
                    nc.vector.tensor_mul(t4[:], st_e[:], s_t)
                    nc.vector.tensor_add(out_o, t3[:], t4[:])

                # --- K pass: kc-outer over 8 psum banks, then rope pairs ---
                kps = [paps.tile([128, Q], F32, name="mmps", tag="mmps")
                       for _ in range(DC)]
                for kc in range(KC):
                    if kc < 2:
                        wkt = wkts[kc]
                    else:
                        wkt = pa.tile([128, D], BF16, name="wk", tag="wk",
                                      bufs=4)
                        nc.sync.dma_start(wkt[:],
                                          wk_d[128 * kc:128 * (kc + 1), :])
                    for dc in range(DC):
                        nc.tensor.matmul(
                            kps[dc][:], wkt[:, 128 * dc:128 * (dc + 1)],
                            hqc[kc // 8][:, kc % 8, :],
                            start=(kc == 0), stop=(kc == KC - 1))
                for pi, (de, do) in enumerate(PAIRS):
                    c_t = pa.tile([128, Q], BF16, name="ckt", tag="ckt",
                                  bufs=2)
                    s_t = pa.tile([128, Q], BF16, name="skt", tag="skt",
                                  bufs=2)
                    nc.sync.dma_start(c_t[:],
                                      cosq_d[128 * pi:128 * (pi + 1), :])
                    nc.sync.dma_start(s_t[:],
                                      sinq_d[128 * pi:128 * (pi + 1), :])
                    ke = pa.tile([128, Q], BF16, name="kout", tag="kout",
                                 bufs=4)
                    ko = pa.tile([128, Q], BF16, name="kout", tag="kout",
                                 bufs=4)
                    rope_pair(kps[de], kps[do], c_t[:], s_t[:], ke[:], ko[:],
                              Q)
                    nc.gpsimd.dma_start(k_in[128 * de:128 * (de + 1), :],
                                        ke[:])
                    nc.gpsimd.dma_start(k_in[128 * do:128 * (do + 1), :],
                                        ko[:])

                # --- k AllGather (overlaps V + Q passes) ---
                nc.gpsimd.collective_compute(
                    "AllGather", mybir.AluOpType.bypass, replica_groups=GROUPS,
                    ins=[k_in.ap().opt()], outs=[k_out.ap().opt()])

                # --- V pass: kc-outer over 8 psum banks ---
                vps = [paps.tile([128, 512], F32, name="mmps", tag="mmps")
                       for _ in range(8)]
                for kc in range(KC):
                    wvt = pa.tile([128, D], BF16, name="wv", tag="wv", bufs=4)
                    nc.sync.dma_start(wvt[:], wv_d[128 * kc:128 * (kc + 1), :])
                    for sc in range(4):
                        for dvb in range(2):
                            nc.tensor.matmul(
                                vps[sc * 2 + dvb][:],
                                hqc[kc // 8][:, kc % 8, 128 * sc:128 * (sc + 1)],
                                wvt[:, 512 * dvb:512 * (dvb + 1)],
                                start=(kc == 0), stop=(kc == KC - 1))
                # hqc2 reuses the hqc ring slots (WAR releases as the V pass
                # drains each hqc tile); the scalar ring is idle here, so
                # these transfers never block the sync (weight) ring
                hqc2 = [pa.tile([128, 8, 512], BF16, name=f"hqc{i}",
                                tag=f"hqc{i}", bufs=1) for i in range(4)]
                for i in range(4):
                    for c in range(8):
                        nc.scalar.dma_start(
                            hqc2[i][:, c, :],
                            hsq2_d[1024 * i + 128 * c:
                                   1024 * i + 128 * (c + 1), :])
                for sc in range(4):
                    for dvb in range(2):
                        vt = pa.tile([128, 512], BF16, name="vout", tag="vout",
                                     bufs=4)
                        nc.scalar.activation(vt[:], vps[sc * 2 + dvb][:],
                                             mybir.ActivationFunctionType.Copy)
                        dst = v_in[256 * sc:256 * (sc + 1), :]
                        dst = dst.rearrange("(p c) f -> p c f", c=2)[:, dvb, :]
                        nc.gpsimd.dma_start(dst, vt[:])

                # --- v AllGather (overlaps Q pass) ---
                nc.gpsimd.collective_compute(
                    "AllGather", mybir.AluOpType.bypass, replica_groups=GROUPS,
                    ins=[v_in.ap().opt()], outs=[v_out.ap().opt()])

                # mask tiles have no collective dependency: safe to enqueue
                for mi in range(n_mixed):
                    nc.gpsimd.dma_start(
                        maskt[mi][:], maskp_d[128 * mi:128 * (mi + 1), :])

                # --- Q pass: all 4 heads x 8 d-chunks over local i cols ---
                # kT/vT loads are emitted mid-Q-pass on the scalar ring: DMA
                # descriptors whose semaphores are far in the future (the
                # AllGathers) head-of-line block the shared DMA engines, so
                # they must not be enqueued until the AGs are ~done.
                for hp in range(4):
                    qps = [paps.tile([128, 512], F32, name="mmps", tag="mmps")
                           for _ in range(DC)]
                    for kc in range(KC):
                        wqt = pa.tile([128, D], BF16, name="wqs", tag="wqs",
                                      bufs=12)
                        nc.sync.dma_start(
                            wqt[:], wq_d[128 * kc:128 * (kc + 1),
                                         D * hp:D * (hp + 1)])
                        for dc in range(DC):
                            nc.tensor.matmul(
                                qps[dc][:], wqt[:, 128 * dc:128 * (dc + 1)],
                                hqc2[kc // 8][:, kc % 8, :],
                                start=(kc == 0), stop=(kc == KC - 1))
                    for pi, (de, do) in enumerate(PAIRS):
                        c_t = pa.tile([128, 512], BF16, name="cqt", tag="cqt",
                                      bufs=3)
                        s_t = pa.tile([128, 512], BF16, name="sqt", tag="sqt",
                                      bufs=3)
                        nc.sync.dma_start(c_t[:],
                                          cosq2_d[128 * pi:128 * (pi + 1), :])
                        nc.sync.dma_start(s_t[:],
                                          sinq2_d[128 * pi:128 * (pi + 1), :])
                        rope_pair(qps[de], qps[do], c_t[:], s_t[:],
                                  qT[DC * hp + de][:], qT[DC * hp + do][:],
                                  512)
                    if hp == 1:
                        for dc in range(DC):
                            for r2 in range(4):
                                nc.scalar.dma_start(
                                    kT[dc][:, Q * r2:Q * (r2 + 1)],
                                    k_out[1024 * r2 + 128 * dc:
                                          1024 * r2 + 128 * (dc + 1), :])
                    elif hp == 2:
                        for jc in range(JC):
                            base = 1024 * (jc // 4) + 256 * (jc % 4)
                            nc.scalar.dma_start(
                                vT[jc][:],
                                v_out[base:base + 256, :].rearrange(
                                    "(p c) f -> p (c f)", c=2))

            # ====== phase B: attention, ragged suffix tiles per jc ======
            # accumulation chains must start with the widest extent so the
            # start matmul zero-initializes every region later accumulated
            chain_jc = sorted(live_jc, key=lambda jc: (slot_start[jc], jc))
            ragged = len({slot_start[jc] for jc in live_jc}) > 1
            with tc.tile_pool(name="pb", bufs=2) as pb:
                pbps_cm = tc.tile_pool(name="pbps", bufs=2, space="PSUM")
                pbps = pbps_cm.__enter__()
                # attention output, transposed: 32 chunks [128 hdv, 512 i]
                attnT = [pb.tile([128, 512], BF16, name=f"att{i}",
                                 tag=f"att{i}", bufs=1) for i in range(KC)]
                if smin > 0:
                    for t in attnT:
                        nc.vector.memset(t[:, :128 * smin], 0.0)

                for h in range(NH):
                    pT = {}
                    for jc in live_jc:
                        s = slot_start[jc]
                        w = 512 - 128 * s
                        sps = pbps.tile([128, 512], F32, name="sps",
                                        tag="sps", bufs=3)
                        for dc in range(DC):
                            nc.tensor.matmul(
                                sps[:, :w],
                                kT[dc][:, 128 * jc:128 * (jc + 1)],
                                qT[DC * h + dc][:, 128 * s:512],
                                start=(dc == 0), stop=(dc == DC - 1))
                        for k in range(s, 4):
                            cat = cats[(jc, k)]
                            if isinstance(cat, int):
                                off = 128 * (k - s)
                                nc.vector.tensor_add(
                                    sps[:, off:off + 128],
                                    sps[:, off:off + 128], maskt[cat][:])
                        pt = pb.tile([128, w], BF16, name=f"pt{jc}",
                                     tag=f"pt{jc}", bufs=2)
                        nc.scalar.activation(
                            pt[:], sps[:, :w],
                            mybir.ActivationFunctionType.Exp, scale=SCALE)
                        pT[jc] = pt
                    # denominator: widest-first chain => shrinking psum extents
                    l_ps = pbps.tile([1, 512], F32, name="lps", tag="lps",
                                     bufs=1)
                    for n, jc in enumerate(chain_jc):
                        s = slot_start[jc]
                        nc.tensor.matmul(l_ps[:, 128 * s:512], ones_col[:],
                                         pT[jc][:], start=(n == 0),
                                         stop=(n == len(chain_jc) - 1),
                                         skip_group_check=ragged)
                    r_sb = pb.tile([1, 512], F32, name="rsb", tag="rsb",
                                   bufs=2)
                    nc.vector.reciprocal(r_sb[:, 128 * smin:512],
                                         l_ps[:, 128 * smin:512])
                    r_ps = pbps.tile([128, 512], F32, name="rps",
                                     tag="rps", bufs=1)
                    nc.tensor.matmul(r_ps[:, 128 * smin:512], ones_row[:],
                                     r_sb[:, 128 * smin:512],
                                     start=True, stop=True)
                    rbc = pb.tile([128, 512], F32, name="rbc", tag="rbc",
                                  bufs=2)
                    nc.scalar.activation(rbc[:, 128 * smin:512],
                                         r_ps[:, 128 * smin:512],
                                         mybir.ActivationFunctionType.Copy)
                    for dc2 in range(DC):
                        pvps = pbps.tile([128, 512], F32, name="pvps",
                                         tag="pvps", bufs=2)
                        for n, jc in enumerate(chain_jc):
                            s = slot_start[jc]
                            nc.tensor.matmul(
                                pvps[:, 128 * s:512],
                                vT[jc][:, 128 * dc2:128 * (dc2 + 1)],
                                pT[jc][:], start=(n == 0),
                                stop=(n == len(chain_jc) - 1),
                                skip_group_check=ragged)
                        nc.vector.tensor_mul(
                            attnT[DC * h + dc2][:, 128 * smin:512],
                            pvps[:, 128 * smin:512],
                            rbc[:, 128 * smin:512])

                pbps_cm.__exit__(None, None, None)

                # ============ phase C: local output projection ============
                with (
                    tc.tile_pool(name="pc", bufs=2) as pc,
                    tc.tile_pool(name="pcps", bufs=4, space="PSUM") as pcps,
                ):
                    for eb in range(8):
                        ops = [pcps.tile([128, 512], F32, name="ops",
                                         tag="ops") for _ in range(4)]
                        for kc in range(KC):
                            wot = pc.tile([128, 512], BF16, name="wot",
                                          tag="wot", bufs=8)
                            nc.sync.dma_start(
                                wot[:], wo_d[128 * kc:128 * (kc + 1),
                                             512 * eb:512 * (eb + 1)])
                            for ic in range(4):
                                nc.tensor.matmul(
                                    ops[ic][:],
                                    attnT[kc][:, 128 * ic:128 * (ic + 1)],
                                    wot[:], start=(kc == 0),
                                    stop=(kc == KC - 1))
                        for ic in range(4):
                            ot = pc.tile([128, 512], F32, name="otile",
                                         tag="otile", bufs=4)
                            nc.vector.tensor_copy(ot[:], ops[ic][:])
                            nc.scalar.dma_start(
                                out_d[128 * ic:128 * (ic + 1),
                                      512 * eb:512 * (eb + 1)], ot[:])

    nc.compile()
    return nc


_BUILD_CACHE = {}


def _classify_mask(mask):
    """Slot-classify each (jc, slot k) over the 4 quarter cores.

    Core r owns global 128-row i-blocks {4k + r}, k in 0..3. Slot k of key
    chunk jc is dead if every core's block is fully masked, clean if every
    core's block is all-zero, else mixed (packed per-core mask tile).
    Returns (slot_start, cats, per-core packed [n*128,128] arrays, n_mixed).
    """
    m = np.asarray(mask).reshape(S, S)  # [i, j]
    slot_start = {}
    cats = {}
    tiles = [[] for _ in range(4)]
    n = 0
    for jc in range(JC):
        regs = {}
        for k in range(4):
            regs[k] = [m[128 * (4 * k + r):128 * (4 * k + r + 1),
                        128 * jc:128 * (jc + 1)] for r in range(4)]
        live_k = [k for k in range(4)
                  if not all(np.all(b <= -1e8) for b in regs[k])]
        if not live_k:
            slot_start[jc] = -1
            continue
        s = min(live_k)
        slot_start[jc] = s
        for k in range(s, 4):
            if not any(b.any() for b in regs[k]):
                cats[(jc, k)] = "clean"
            else:
                cats[(jc, k)] = n
                n += 1
                for r in range(4):
                    # [j, i] orientation, prescaled by 1/SCALE so the ACT's
                    # uniform SCALE reproduces reference's scores*SCALE + mask
                    tiles[r].append(
                        np.ascontiguousarray(regs[k][r].T) * (1.0 / SCALE))
    maskps = [
        np.concatenate(t, axis=0).astype(np.float32) if t
        else np.zeros((128, 128), np.float32) for t in tiles]
    return slot_start, cats, maskps, n


def kernel(hidden_states, attention_mask, Wq, Wk, Wv, Wo, trace=False):
    global LAST_RESULTS
    bf = ml_dtypes.bfloat16

    slot_start, cats, maskps, n_mixed = _classify_mask(attention_mask)
    key = (tuple(sorted(slot_start.items())),
           tuple(sorted((k, v if isinstance(v, str) else "m")
                        for k, v in cats.items())))
    if key not in _BUILD_CACHE:
        _BUILD_CACHE[key] = _build(slot_start, cats, n_mixed)
    nc = _BUILD_CACHE[key]

    # deinterleave rope pairs within each head's 1024 columns
    perm = np.concatenate([np.arange(0, D, 2), np.arange(1, D, 2)])
    cols = np.concatenate([h * D + perm for h in range(NH)])
    wq_p = np.ascontiguousarray(Wq[:, cols]).astype(bf)
    wk_p = np.ascontiguousarray(Wk[:, perm]).astype(bf)
    wv_c = np.ascontiguousarray(Wv).astype(bf)
    wo_c = np.ascontiguousarray(Wo).astype(bf)

    freqs = 1.0 / (10000.0 ** (np.arange(0, D, 2, dtype=np.float64) / D))
    ang = np.outer(np.arange(S, dtype=np.float64), freqs)  # [S, PD]
    cosT = np.ascontiguousarray(np.cos(ang).T).astype(bf)  # [PD, S]
    sinT = np.ascontiguousarray(np.sin(ang).T).astype(bf)

    hsT = [np.ascontiguousarray(hidden_states[b].T).astype(bf)
           for b in range(B)]

    in_maps = []
    for c in range(NCORES):
        b, r = c // 4, c % 4
        icols = np.concatenate(
            [np.arange(128 * (4 * k + r), 128 * (4 * k + r + 1))
             for k in range(4)])
        in_maps.append({
            "hsq": np.ascontiguousarray(hsT[b][:, Q * r:Q * (r + 1)]),
            "hsq2": np.ascontiguousarray(hsT[b][:, icols]),
            "wq": wq_p,
            "wk": wk_p,
            "wv": wv_c,
            "wo": wo_c,
            "cosq": np.ascontiguousarray(cosT[:, Q * r:Q * (r + 1)]),
            "sinq": np.ascontiguousarray(sinT[:, Q * r:Q * (r + 1)]),
            "cosq2": np.ascontiguousarray(cosT[:, icols]),
            "sinq2": np.ascontiguousarray(sinT[:, icols]),
            "maskp": maskps[r],
        })

    res = bass_utils.run_bass_kernel_spmd(
        nc, in_maps, core_ids=list(range(NCORES)), trace=trace)
    LAST_RESULTS = res

    out = np.empty((B, S, H), np.float32)
    for c in range(NCORES):
        b, r = c // 4, c % 4
        o = res.results[c]["out"]
        for k in range(4):
            g = 4 * k + r
            out[b, 128 * g:128 * (g + 1), :] = o[128 * k:128 * (k + 1)]
    return out


# revision 21
# speedup vs baseline: 1.0749x; 1.0129x over previous
"""Multi-head latent attention kernel for Trainium2, 8 NeuronCores.

Problem (hardcoded shapes):
  hidden_states [2, 2048, 4096] f32, attention_mask [1,1,2048,2048] f32,
  Wq [4096,4096], Wk/Wv [4096,1024], Wo [4096,4096].
  4 query heads x 1024 head_dim, 1 kv head, interleaved RoPE, softmax, o-proj.

Sharding: core c = (batch b=c//4, r=c%4), all within-batch groups of 4.
  - k^T / v computed from the core's sequence quarter and AllGathered in two
    separate collectives (k right after the K pass, v after the V pass) so
    both overlap the Q projection.
  - Attention is sharded over QUERY positions at 128-row granularity: core r
    handles global 128-row i-blocks {4k + r} for slot k in 0..3 -- slot k's
    union key range is jc < 4k+4, identical for every core, so the SPMD
    program wastes only the 128-col mask wedge instead of whole 256-row
    blocks. Scores/exp/PV use ragged suffix tiles: for key chunk jc the
    moving operand is qT[:, 128*s(jc):512] (s = first live slot), and the
    denominator / PV accumulations walk jc ascending so PSUM regions shrink
    inside one accumulation group (first matmul zero-initializes the full
    bank). The o-projection stays fully local (no second collective).
  - DMA queue discipline: weights (wk/wv/wq/wo) + cos/sin + initial hsq
    chunks ride the Sync ring; hqc2 (WAR-gated reloads) and output stores
    ride the Vector ring; k/v bounce stores, both AllGathers, kT/vT/mask
    loads ride the GpSimd ring. No consumer ever queues behind a slow
    producer on the same FIFO ring.
All matmul operands bf16, f32 PSUM accumulation. RoPE is a host-side
deinterleave permutation of Wq/Wk columns plus 6 elementwise ops per
(even,odd) chunk pair against cos/sin tables. The mask is handled
generically per (jc, slot): skip / clean / mixed with per-core
(1/SCALE)-prescaled 128x128 mask tiles, so causal, zero, and arbitrary
additive masks are all supported.
"""

import numpy as np
import ml_dtypes

from concourse import bass, mybir, tile, bacc
from concourse import bass_utils

BF16 = mybir.dt.bfloat16
F32 = mybir.dt.float32

B, S, H = 2, 2048, 4096
NH, D = 4, 1024  # query heads, head dim
PD = D // 2  # rope pair count (512)
SCALE = D ** -0.5
NCORES = 8
GROUPS = [[0, 1, 2, 3], [4, 5, 6, 7]]

KC = H // 128  # 32 contraction chunks over hidden
DC = D // 128  # 8 d-chunks of head dim
JC = S // 128  # 16 j-chunks (key) of 128
Q = S // 4  # 512, per-core kv sequence quarter

# results of the traced+profiled run (filled by kernel() when trace=True)
LAST_RESULTS = None

PAIRS = [(0, 4), (1, 5), (2, 6), (3, 7)]


def _build(slot_start, cats, n_mixed):
    """Build the SPMD bass program.

    slot_start[jc]: first live slot s for key chunk jc, or -1 to skip jc.
    cats[(jc, k)] for k in s(jc)..3: "clean" | int (packed mask tile index).
    """
    nc = bacc.Bacc("TRN2", target_bir_lowering=False, debug=False,
                   num_devices=NCORES)

    hsq_d = nc.dram_tensor("hsq", [H, Q], BF16, kind="ExternalInput")
    hsq2_d = nc.dram_tensor("hsq2", [H, 512], BF16, kind="ExternalInput")
    wq_d = nc.dram_tensor("wq", [H, H], BF16, kind="ExternalInput")
    wk_d = nc.dram_tensor("wk", [H, D], BF16, kind="ExternalInput")
    wv_d = nc.dram_tensor("wv", [H, D], BF16, kind="ExternalInput")
    wo_d = nc.dram_tensor("wo", [H, H], BF16, kind="ExternalInput")
    cosq_d = nc.dram_tensor("cosq", [PD, Q], BF16, kind="ExternalInput")
    sinq_d = nc.dram_tensor("sinq", [PD, Q], BF16, kind="ExternalInput")
    cosq2_d = nc.dram_tensor("cosq2", [PD, 512], BF16, kind="ExternalInput")
    sinq2_d = nc.dram_tensor("sinq2", [PD, 512], BF16, kind="ExternalInput")
    nmask = max(n_mixed, 1)
    maskp_d = nc.dram_tensor("maskp", [nmask * 128, 128], F32,
                             kind="ExternalInput")
    out_d = nc.dram_tensor("out", [512, H], F32, kind="ExternalOutput")

    # combined collective bounce: rows [0:1024] = k^T slice [1024 d, 512 s],
    # rows [1024:2048] = v slice flattened row-major (dram row 1024+2p+dvb =
    # s-row p, d half dvb)
    kv_in = nc.dram_tensor("kv_in", [2048, 512], BF16, kind="Internal")
    kv_out = nc.dram_tensor("kv_out", [4 * 2048, 512], BF16, kind="Internal")

    live_jc = [jc for jc in range(JC) if slot_start[jc] >= 0]
    smin = min(slot_start[jc] for jc in live_jc) if live_jc else 0

    with tile.TileContext(nc) as tc:
        with tc.tile_pool(name="pers", bufs=1) as pers:
            ones_col = pers.tile([128, 1], BF16, name="ones_col",
                                 tag="ones_col")
            nc.vector.memset(ones_col[:], 1.0)
            ones_row = pers.tile([1, 128], F32, name="ones_row",
                                 tag="ones_row")
            nc.vector.memset(ones_row[:], 1.0)
            # q^T for all 4 heads, local i columns: 32 chunks [128 d, 512 i]
            qT = [pers.tile([128, 512], BF16, name=f"qt{i}", tag=f"qt{i}")
                  for i in range(4 * DC)]
            # k^T full sequence: 8 chunks [128 d, 2048 j]
            kT = [pers.tile([128, S], BF16, name=f"kt{i}", tag=f"kt{i}")
                  for i in range(DC)]
            # v rows per 128-key chunk: 16 tiles [128 j, 1024 d]
            vT = [pers.tile([128, D], BF16, name=f"vt{i}", tag=f"vt{i}")
                  for i in range(JC)]
            maskt = [pers.tile([128, 128], F32, name=f"mk{i}", tag=f"mk{i}")
                     for i in range(n_mixed)]

            # ============ phase A: projections + k/v AllGathers ============
            with (
                tc.tile_pool(name="pa", bufs=3) as pa,
                tc.tile_pool(name="paps", bufs=8, space="PSUM") as paps,
            ):
                hqc = [pa.tile([128, 8, Q], BF16, name=f"hqc{i}",
                               tag=f"hqc{i}", bufs=1) for i in range(4)]

                def rope_pair(ps_e, ps_o, c_t, s_t, out_e, out_o, n):
                    """Stage psum pair to bf16, apply rope, write outputs."""
                    st_e = pa.tile([128, n], BF16, name="stg", tag="stg",
                                   bufs=6)
                    st_o = pa.tile([128, n], BF16, name="stg", tag="stg",
                                   bufs=6)
                    nc.scalar.activation(st_e[:], ps_e[:],
                                         mybir.ActivationFunctionType.Copy)
                    nc.scalar.activation(st_o[:], ps_o[:],
                                         mybir.ActivationFunctionType.Copy)
                    t1 = pa.tile([128, n], BF16, name="rtmp", tag="rtmp",
                                 bufs=4)
                    t2 = pa.tile([128, n], BF16, name="rtmp", tag="rtmp",
                                 bufs=4)
                    nc.vector.tensor_mul(t1[:], st_e[:], c_t)
                    nc.vector.tensor_mul(t2[:], st_o[:], s_t)
                    nc.vector.tensor_sub(out_e, t1[:], t2[:])
                    t3 = pa.tile([128, n], BF16, name="rtmp", tag="rtmp",
                                 bufs=4)
                    t4 = pa.tile([128, n], BF16, name="rtmp", tag="rtmp",
                                 bufs=4)
                    nc.vector.tensor_mul(t3[:], st_o[:], c_t)
                    nc.vector.tensor_mul(t4[:], st_e[:], s_t)
                    nc.vector.tensor_add(out_o, t3[:], t4[:])

                # --- K pass: kc-outer over 8 psum banks, then rope pairs ---
                # wk tiles and hqc chunks alternate on the ring so the first
                # matmul starts after ~0.4MB and the stream self-paces
                kps = [paps.tile([128, Q], F32, name="mmps", tag="mmps")
                       for _ in range(DC)]
                for kc in range(KC):
                    wkt = pa.tile([128, D], BF16, name="wk", tag="wk",
                                  bufs=5)
                    nc.sync.dma_start(wkt[:],
                                      wk_d[128 * kc:128 * (kc + 1), :])
                    nc.sync.dma_start(
                        hqc[kc // 8][:, kc % 8, :],
                        hsq_d[128 * kc:128 * (kc + 1), :])
                    for dc in range(DC):
                        nc.tensor.matmul(
                            kps[dc][:], wkt[:, 128 * dc:128 * (dc + 1)],
                            hqc[kc // 8][:, kc % 8, :],
                            start=(kc == 0), stop=(kc == KC - 1))
                for pi, (de, do) in enumerate(PAIRS):
                    c_t = pa.tile([128, Q], BF16, name="ckt", tag="ckt",
                                  bufs=2)
                    s_t = pa.tile([128, Q], BF16, name="skt", tag="skt",
                                  bufs=2)
                    nc.sync.dma_start(c_t[:],
                                      cosq_d[128 * pi:128 * (pi + 1), :])
                    nc.sync.dma_start(s_t[:],
                                      sinq_d[128 * pi:128 * (pi + 1), :])
                    ke = pa.tile([128, Q], BF16, name="kout", tag="kout",
                                 bufs=4)
                    ko = pa.tile([128, Q], BF16, name="kout", tag="kout",
                                 bufs=4)
                    rope_pair(kps[de], kps[do], c_t[:], s_t[:], ke[:], ko[:],
                              Q)
                    nc.scalar.dma_start(kv_in[128 * de:128 * (de + 1), :],
                                        ke[:])
                    nc.scalar.dma_start(kv_in[128 * do:128 * (do + 1), :],
                                        ko[:])

                # --- V pass: kc-outer over 8 psum banks ---
                vps = [paps.tile([128, 512], F32, name="mmps", tag="mmps")
                       for _ in range(8)]
                for kc in range(KC):
                    wvt = pa.tile([128, D], BF16, name="wv", tag="wv", bufs=4)
                    nc.sync.dma_start(wvt[:], wv_d[128 * kc:128 * (kc + 1), :])
                    for sc in range(4):
                        for dvb in range(2):
                            nc.tensor.matmul(
                                vps[sc * 2 + dvb][:],
                                hqc[kc // 8][:, kc % 8, 128 * sc:128 * (sc + 1)],
                                wvt[:, 512 * dvb:512 * (dvb + 1)],
                                start=(kc == 0), stop=(kc == KC - 1))
                # hqc2 reuses the hqc ring slots (WAR releases as the V pass
                # drains each hqc tile); the scalar ring is idle here, so
                # these transfers never block the sync (weight) ring
                hqc2 = [pa.tile([128, 8, 512], BF16, name=f"hqc{i}",
                                tag=f"hqc{i}", bufs=1) for i in range(4)]
                for i in range(4):
                    for c in range(8):
                        nc.scalar.dma_start(
                            hqc2[i][:, c, :],
                            hsq2_d[1024 * i + 128 * c:
                                   1024 * i + 128 * (c + 1), :])
                for sc in range(4):
                    for dvb in range(2):
                        vt = pa.tile([128, 512], BF16, name="vout", tag="vout",
                                     bufs=4)
                        nc.scalar.activation(vt[:], vps[sc * 2 + dvb][:],
                                             mybir.ActivationFunctionType.Copy)
                        dst = kv_in[1024 + 256 * sc:1024 + 256 * (sc + 1), :]
                        dst = dst.rearrange("(p c) f -> p c f", c=2)[:, dvb, :]
                        nc.scalar.dma_start(dst, vt[:])

                # --- combined kv AllGather (overlaps Q pass); one gpsimd
                # trigger instruction, no slow gpsimd dma_start chain ---
                nc.gpsimd.collective_compute(
                    "AllGather", mybir.AluOpType.bypass, replica_groups=GROUPS,
                    ins=[kv_in.ap().opt()], outs=[kv_out.ap().opt()])

                # mask tiles have no collective dependency: safe to enqueue
                for mi in range(n_mixed):
                    nc.gpsimd.dma_start(
                        maskt[mi][:], maskp_d[128 * mi:128 * (mi + 1), :])

                # --- Q pass: all 4 heads x 8 d-chunks over local i cols ---
                # kT/vT loads are emitted mid-Q-pass on the scalar ring: DMA
                # descriptors whose semaphores are far in the future (the
                # AllGathers) head-of-line block the shared DMA engines, so
                # they must not be enqueued until the AGs are ~done.
                for hp in range(4):
                    qps = [paps.tile([128, 512], F32, name="mmps", tag="mmps")
                           for _ in range(DC)]
                    for kc in range(KC):
                        # alternate the weight stream over two DMA rings so
                        # it keeps up while the AllGather eats bandwidth
                        wqt = pa.tile([128, D], BF16, name="wqs", tag="wqs",
                                      bufs=12)
                        eng = nc.sync if kc % 2 == 0 else nc.scalar
                        eng.dma_start(
                            wqt[:], wq_d[128 * kc:128 * (kc + 1),
                                         D * hp:D * (hp + 1)])
                        for dc in range(DC):
                            nc.tensor.matmul(
                                qps[dc][:], wqt[:, 128 * dc:128 * (dc + 1)],
                                hqc2[kc // 8][:, kc % 8, :],
                                start=(kc == 0), stop=(kc == KC - 1))
                    for pi, (de, do) in enumerate(PAIRS):
                        c_t = pa.tile([128, 512], BF16, name="cqt", tag="cqt",
                                      bufs=3)
                        s_t = pa.tile([128, 512], BF16, name="sqt", tag="sqt",
                                      bufs=3)
                        nc.sync.dma_start(c_t[:],
                                          cosq2_d[128 * pi:128 * (pi + 1), :])
                        nc.sync.dma_start(s_t[:],
                                          sinq2_d[128 * pi:128 * (pi + 1), :])
                        rope_pair(qps[de], qps[do], c_t[:], s_t[:],
                                  qT[DC * hp + de][:], qT[DC * hp + do][:],
                                  512)
                    if hp == 2:
                        for dc in range(DC):
                            for r2 in range(4):
                                nc.scalar.dma_start(
                                    kT[dc][:, Q * r2:Q * (r2 + 1)],
                                    kv_out[2048 * r2 + 128 * dc:
                                           2048 * r2 + 128 * (dc + 1), :])
                    elif hp == 3:
                        for jc in range(JC):
                            base = 2048 * (jc // 4) + 1024 + 256 * (jc % 4)
                            nc.scalar.dma_start(
                                vT[jc][:],
                                kv_out[base:base + 256, :].rearrange(
                                    "(p c) f -> p (c f)", c=2))

            # ====== phase B: attention, ragged suffix tiles per jc ======
            # accumulation chains must start with the widest extent so the
            # start matmul zero-initializes every region later accumulated
            chain_jc = sorted(live_jc, key=lambda jc: (slot_start[jc], jc))
            ragged = len({slot_start[jc] for jc in live_jc}) > 1
            with tc.tile_pool(name="pb", bufs=2) as pb:
                pbps_cm = tc.tile_pool(name="pbps", bufs=2, space="PSUM")
                pbps = pbps_cm.__enter__()
                # attention output, transposed: 32 chunks [128 hdv, 512 i]
                attnT = [pb.tile([128, 512], BF16, name=f"att{i}",
                                 tag=f"att{i}", bufs=1) for i in range(KC)]
                if smin > 0:
                    for t in attnT:
                        nc.vector.memset(t[:, :128 * smin], 0.0)

                for h in range(NH):
                    pT = {}
                    for jc in live_jc:
                        s = slot_start[jc]
                        w = 512 - 128 * s
                        sps = pbps.tile([128, 512], F32, name="sps",
                                        tag="sps", bufs=3)
                        for dc in range(DC):
                            nc.tensor.matmul(
                                sps[:, :w],
                                kT[dc][:, 128 * jc:128 * (jc + 1)],
                                qT[DC * h + dc][:, 128 * s:512],
                                start=(dc == 0), stop=(dc == DC - 1))
                        for k in range(s, 4):
                            cat = cats[(jc, k)]
                            if isinstance(cat, int):
                                off = 128 * (k - s)
                                nc.vector.tensor_add(
                                    sps[:, off:off + 128],
                                    sps[:, off:off + 128], maskt[cat][:])
                        pt = pb.tile([128, w], BF16, name=f"pt{jc}",
                                     tag=f"pt{jc}", bufs=2)
                        nc.scalar.activation(
                            pt[:], sps[:, :w],
                            mybir.ActivationFunctionType.Exp, scale=SCALE)
                        pT[jc] = pt
                    # denominator: widest-first chain => shrinking psum extents
                    l_ps = pbps.tile([1, 512], F32, name="lps", tag="lps",
                                     bufs=1)
                    for n, jc in enumerate(chain_jc):
                        s = slot_start[jc]
                        nc.tensor.matmul(l_ps[:, 128 * s:512], ones_col[:],
                                         pT[jc][:], start=(n == 0),
                                         stop=(n == len(chain_jc) - 1),
                                         skip_group_check=ragged)
                    r_sb = pb.tile([1, 512], F32, name="rsb", tag="rsb",
                                   bufs=2)
                    nc.vector.reciprocal(r_sb[:, 128 * smin:512],
                                         l_ps[:, 128 * smin:512])
                    r_ps = pbps.tile([128, 512], F32, name="rps",
                                     tag="rps", bufs=1)
                    nc.tensor.matmul(r_ps[:, 128 * smin:512], ones_row[:],
                                     r_sb[:, 128 * smin:512],
                                     start=True, stop=True)
                    rbc = pb.tile([128, 512], F32, name="rbc", tag="rbc",
                                  bufs=2)
                    nc.scalar.activation(rbc[:, 128 * smin:512],
                                         r_ps[:, 128 * smin:512],
                                         mybir.ActivationFunctionType.Copy)
                    for dc2 in range(DC):
                        pvps = pbps.tile([128, 512], F32, name="pvps",
                                         tag="pvps", bufs=2)
                        for n, jc in enumerate(chain_jc):
                            s = slot_start[jc]
                            nc.tensor.matmul(
                                pvps[:, 128 * s:512],
                                vT[jc][:, 128 * dc2:128 * (dc2 + 1)],
                                pT[jc][:], start=(n == 0),
                                stop=(n == len(chain_jc) - 1),
                                skip_group_check=ragged)
                        nc.vector.tensor_mul(
                            attnT[DC * h + dc2][:, 128 * smin:512],
                            pvps[:, 128 * smin:512],
                            rbc[:, 128 * smin:512])

                pbps_cm.__exit__(None, None, None)

                # ============ phase C: local output projection ============
                with (
                    tc.tile_pool(name="pc", bufs=2) as pc,
                    tc.tile_pool(name="pcps", bufs=4, space="PSUM") as pcps,
                ):
                    for eb in range(8):
                        ops = [pcps.tile([128, 512], F32, name="ops",
                                         tag="ops") for _ in range(4)]
                        for kc in range(KC):
                            wot = pc.tile([128, 512], BF16, name="wot",
                                          tag="wot", bufs=8)
                            nc.sync.dma_start(
                                wot[:], wo_d[128 * kc:128 * (kc + 1),
                                             512 * eb:512 * (eb + 1)])
                            for ic in range(4):
                                nc.tensor.matmul(
                                    ops[ic][:],
                                    attnT[kc][:, 128 * ic:128 * (ic + 1)],
                                    wot[:], start=(kc == 0),
                                    stop=(kc == KC - 1))
                        for ic in range(4):
                            ot = pc.tile([128, 512], F32, name="otile",
                                         tag="otile", bufs=4)
                            nc.vector.tensor_copy(ot[:], ops[ic][:])
                            nc.scalar.dma_start(
                                out_d[128 * ic:128 * (ic + 1),
                                      512 * eb:512 * (eb + 1)], ot[:])

    nc.compile()
    return nc


_BUILD_CACHE = {}


def _classify_mask(mask):
    """Slot-classify each (jc, slot k) over the 4 quarter cores.

    Core r owns global 128-row i-blocks {4k + r}, k in 0..3. Slot k of key
    chunk jc is dead if every core's block is fully masked, clean if every
    core's block is all-zero, else mixed (packed per-core mask tile).
    Returns (slot_start, cats, per-core packed [n*128,128] arrays, n_mixed).
    """
    m = np.asarray(mask).reshape(S, S)  # [i, j]
    slot_start = {}
    cats = {}
    tiles = [[] for _ in range(4)]
    n = 0
    for jc in range(JC):
        regs = {}
        for k in range(4):
            regs[k] = [m[128 * (4 * k + r):128 * (4 * k + r + 1),
                        128 * jc:128 * (jc + 1)] for r in range(4)]
        live_k = [k for k in range(4)
                  if not all(np.all(b <= -1e8) for b in regs[k])]
        if not live_k:
            slot_start[jc] = -1
            continue
        s = min(live_k)
        slot_start[jc] = s
        for k in range(s, 4):
            if not any(b.any() for b in regs[k]):
                cats[(jc, k)] = "clean"
            else:
                cats[(jc, k)] = n
                n += 1
                for r in range(4):
                    # [j, i] orientation, prescaled by 1/SCALE so the ACT's
                    # uniform SCALE reproduces reference's scores*SCALE + mask
                    tiles[r].append(
                        np.ascontiguousarray(regs[k][r].T) * (1.0 / SCALE))
    maskps = [
        np.concatenate(t, axis=0).astype(np.float32) if t
        else np.zeros((128, 128), np.float32) for t in tiles]
    return slot_start, cats, maskps, n


def kernel(hidden_states, attention_mask, Wq, Wk, Wv, Wo, trace=False):
    global LAST_RESULTS
    bf = ml_dtypes.bfloat16

    slot_start, cats, maskps, n_mixed = _classify_mask(attention_mask)
    key = (tuple(sorted(slot_start.items())),
           tuple(sorted((k, v if isinstance(v, str) else "m")
                        for k, v in cats.items())))
    if key not in _BUILD_CACHE:
        _BUILD_CACHE[key] = _build(slot_start, cats, n_mixed)
    nc = _BUILD_CACHE[key]

    # deinterleave rope pairs within each head's 1024 columns
    perm = np.concatenate([np.arange(0, D, 2), np.arange(1, D, 2)])
    cols = np.concatenate([h * D + perm for h in range(NH)])
    wq_p = np.ascontiguousarray(Wq[:, cols]).astype(bf)
    wk_p = np.ascontiguousarray(Wk[:, perm]).astype(bf)
    wv_c = np.ascontiguousarray(Wv).astype(bf)
    wo_c = np.ascontiguousarray(Wo).astype(bf)

    freqs = 1.0 / (10000.0 ** (np.arange(0, D, 2, dtype=np.float64) / D))
    ang = np.outer(np.arange(S, dtype=np.float64), freqs)  # [S, PD]
    cosT = np.ascontiguousarray(np.cos(ang).T).astype(bf)  # [PD, S]
    sinT = np.ascontiguousarray(np.sin(ang).T).astype(bf)

    hsT = [np.ascontiguousarray(hidden_states[b].T).astype(bf)
           for b in range(B)]

    in_maps = []
    for c in range(NCORES):
        b, r = c // 4, c % 4
        icols = np.concatenate(
            [np.arange(128 * (4 * k + r), 128 * (4 * k + r + 1))
             for k in range(4)])
        in_maps.append({
            "hsq": np.ascontiguousarray(hsT[b][:, Q * r:Q * (r + 1)]),
            "hsq2": np.ascontiguousarray(hsT[b][:, icols]),
            "wq": wq_p,
            "wk": wk_p,
            "wv": wv_c,
            "wo": wo_c,
            "cosq": np.ascontiguousarray(cosT[:, Q * r:Q * (r + 1)]),
            "sinq": np.ascontiguousarray(sinT[:, Q * r:Q * (r + 1)]),
            "cosq2": np.ascontiguousarray(cosT[:, icols]),
            "sinq2": np.ascontiguousarray(sinT[:, icols]),
            "maskp": maskps[r],
        })

    res = bass_utils.run_bass_kernel_spmd(
        nc, in_maps, core_ids=list(range(NCORES)), trace=trace)
    LAST_RESULTS = res

    out = np.empty((B, S, H), np.float32)
    for c in range(NCORES):
        b, r = c // 4, c % 4
        o = res.results[c]["out"]
        for k in range(4):
            g = 4 * k + r
            out[b, 128 * g:128 * (g + 1), :] = o[128 * k:128 * (k + 1)]
    return out
